# revision 6
# baseline (speedup 1.0000x reference)
"""Trainium2 Bass kernel for a CustomGRUCell.

reference:
    r = sigmoid(x @ W_ir.T + b_ir + h @ W_hr.T)
    z = sigmoid(x @ W_iz.T + b_iz + h @ W_hz.T)
    n = tanh(x @ W_in.T + b_in + (r * h) @ W_hn.T)
    h_t = (1 - z) * n + z * h
    returns (h_t, r, z, n)

Shapes: x,h [8192, 2048]; W_* [2048, 2048]; b_* [2048]. All float32.

Strategy: data-parallel over the batch dim (1024 rows per core, 8 cores),
weights replicated. All compute happens in the "transposed world":
the host packs x^T, h^T and W^T so the contraction dim (IN / H-col) lands
on SBUF partitions for both matmul operands; outputs come back as
gate^T [H, B_shard] and are untransposed on the host.

All matmul operands are float16 (same 1 cycle/row PE rate as fp32r at
moving>=256, half the HBM traffic, and a 10-bit mantissa that keeps the
quantization error ~2e-4 rel). PSUM accumulates fp32. x^T, h^T, r*h and
n all stay resident in SBUF (fp16 halves their footprint), so there are
no DRAM scratch roundtrips at all. Outputs are written fp16 and upcast
to fp32 on the host.

Per-core device schedule (M-tile = 128 rows of H, N chunk = 512 batch
cols, K subtile = 128):
  warmup:      short dummy-matmul accumulation chain issued before any
               input DMA lands, so the PE p-state ramp happens while
               waiting for data instead of during the first real tiles.
  phase 1 (r): psum = sum_k W_ir^T[k,m] x^T[k,n] + sum_k W_hr^T h^T
               r = sigmoid(psum + b_ir)  -> DRAM (fp16)
               rh = r * h^T              -> rh_sb (SBUF, fp16)
  phase 2 (n): psum = x-gemm + rh-gemm; n = tanh(psum + b_in)
               -> n_sb (SBUF, fp16) -> DRAM (fp16)
  phase 3 (z): psum = x-gemm + h-gemm;  z = sigmoid(psum + b_iz) -> DRAM
               h_t = n + z*(h - n)       -> DRAM (all operands in SBUF)
"""

import numpy as np

import concourse.bass as bass
import concourse.bacc as bacc
import concourse.mybir as mybir
import concourse.tile as tile
from concourse.bass_utils import run_bass_kernel_spmd

F32 = mybir.dt.float32
F16 = mybir.dt.float16
AFT = mybir.ActivationFunctionType

# Problem constants (hardcoded per contract).
B_FULL = 8192
IN = 2048
H = 2048
N_CORES = 8
BS = B_FULL // N_CORES  # 1024 batch rows per core
P = 128
KO_IN = IN // P  # 16 contraction subtiles for x-gemms
KO_H = H // P    # 16 contraction subtiles for h/rh-gemms
MT = H // P      # 16 output row tiles
NFREE = 512      # moving free dim per matmul (1 PSUM bank of fp32)
NB = BS // NFREE  # 2 batch chunks per core
WARM_MMS = 7     # dummy matmuls to ride out the PE p-state ramp

# Set by the test harness to capture an NTFF profile.
TRACE = False
LAST_RESULTS = None


def _build_nc():
    nc = bacc.Bacc("TRN2", target_bir_lowering=False, debug=False)

    xT = nc.dram_tensor("xT", [P, KO_IN, BS], F16, kind="ExternalInput").ap()
    hT = nc.dram_tensor("hT", [P, KO_H, BS], F16, kind="ExternalInput").ap()
    w = {
        name: nc.dram_tensor(name, [MT, P, KO_IN * P], F16, kind="ExternalInput").ap()
        for name in ("w_ir", "w_hr", "w_iz", "w_hz", "w_in", "w_hn")
    }
    b = {
        name: nc.dram_tensor(name, [P, MT], F32, kind="ExternalInput").ap()
        for name in ("b_ir", "b_iz", "b_in")
    }
    outs = {
        name: nc.dram_tensor(name, [MT, P, BS], F16, kind="ExternalOutput").ap()
        for name in ("rT", "zT", "nT", "htT")
    }

    with tile.TileContext(nc) as tc:
        with (
            tc.tile_pool(name="xres", bufs=1) as x_pool,
            tc.tile_pool(name="hres", bufs=1) as h_pool,
            tc.tile_pool(name="rhres", bufs=1) as rh_pool,
            tc.tile_pool(name="nres", bufs=1) as n_pool,
            tc.tile_pool(name="warm", bufs=1) as warm_pool,
            tc.tile_pool(name="wstream", bufs=8) as w_pool,
            tc.tile_pool(name="gates", bufs=8) as g_pool,
            tc.tile_pool(name="bias", bufs=1) as b_pool,
            tc.tile_pool(name="psum", bufs=8, space="PSUM") as ps_pool,
        ):
            # PE warmup: a dummy accumulation chain on zeroed tiles keeps
            # the tensor engine continuously busy through its p-state ramp
            # while the first input DMAs are still in flight.
            warm_l = warm_pool.tile([P, P], F16, tag="wl")
            warm_r = warm_pool.tile([P, NFREE], F16, tag="wr")
            nc.vector.memset(warm_l[:], 0.0)
            nc.vector.memset(warm_r[:], 0.0)
            warm_ps = ps_pool.tile([P, NFREE], F32, tag="ps", name="ps_warm")
            for i in range(WARM_MMS):
                nc.tensor.matmul(
                    warm_ps[:], warm_l[:], warm_r[:],
                    start=(i == 0), stop=(i == WARM_MMS - 1),
                )

            BLK = 3  # m-tiles in the streaming head block of each phase

            def w_tile(w_ap, mt, nm, eng=None):
                t = w_pool.tile([P, KO_IN * P], F16, tag="w", name=nm)
                (eng or nc.sync).dma_start(t[:], w_ap[mt])
                return t

            # x^T / h^T stay resident in SBUF for all three phases.
            # Startup DMAs are fanned out across four engine dispatch
            # queues (each DMA_DIRECT2D dispatch costs ~600ns serially on
            # its queue): sync gets x0 + remaining head weights, gpsimd
            # gets wa0/h and the x stream, vector gets wb0, scalar gets the
            # biases. This puts the first A-side matmul's operands (x0,
            # wa0) at the very front of two different queues.
            x_sb = x_pool.tile([P, KO_IN, BS], F16, tag="x")
            h_sb = h_pool.tile([P, KO_H, BS], F16, tag="h")
            rh_sb = rh_pool.tile([P, KO_H, BS], F16, tag="rh")
            n_sb = n_pool.tile([P, KO_H, BS], F16, tag="n")

            nc.sync.dma_start(x_sb[:, 0, :], xT[:, 0, :])
            pre1a, pre1b = {}, {}
            pre1a[0] = w_tile(w["w_ir"], 0, "wa0", eng=nc.gpsimd)
            pre1b[0] = w_tile(w["w_hr"], 0, "wb0", eng=nc.scalar)
            nc.gpsimd.dma_start(h_sb[:, 0, :], hT[:, 0, :])
            for ko in range(1, BLK):
                pre1a[ko] = w_tile(w["w_ir"], ko, f"wa{ko}")
                pre1b[ko] = w_tile(w["w_hr"], ko, f"wb{ko}")
            for ko in range(1, KO_IN):
                nc.gpsimd.dma_start(x_sb[:, ko, :], xT[:, ko, :])
                nc.gpsimd.dma_start(h_sb[:, ko, :], hT[:, ko, :])

            bias_sb = {}
            for name in ("b_ir", "b_iz", "b_in"):
                t = b_pool.tile([P, MT], F32, tag=name)
                nc.scalar.dma_start(t[:], b[name][:])
                bias_sb[name] = t

            def phase(wa_ap, wb_ap, rhs_a, rhs_b, consume, interleave_ab,
                      preload=None):
                """Head block (first BLK m-tiles): ko-loop OUTER so the PE
                consumes streaming rhs tiles in DMA arrival order (kills
                startup / phase-transition stalls). interleave_ab pairs A/B
                at each ko (phase 1: x and h arrive interleaved); otherwise
                all A first (B source still being produced at phase start).
                Remaining m-tiles: mt-wise with sequential per-bank
                accumulation for smooth PSUM handoff."""
                mts = list(range(0, BLK))
                units = [(mt, nb) for mt in mts for nb in range(NB)]
                wa, wb = preload if preload else ({}, {})
                ps = {
                    u: ps_pool.tile(
                        [P, NFREE], F32, tag="ps", name=f"ps_{u[0]}_{u[1]}")
                    for u in units
                }

                def mm(u, w_t, rhs, ko, start, stop):
                    mt, nb = u
                    nc.tensor.matmul(
                        ps[u][:],
                        w_t[mt][:, ko * P:(ko + 1) * P],
                        rhs[:, ko, nb * NFREE:(nb + 1) * NFREE],
                        start=start,
                        stop=stop,
                    )

                if interleave_ab:
                    # mt-major with A/B adjacent: matches the DMA arrival
                    # order x_ko,wa,wb,h_ko at startup
                    for ko in range(KO_IN):
                        for mt in mts:
                            for nb in range(NB):
                                mm((mt, nb), wa, rhs_a, ko, ko == 0, False)
                            for nb in range(NB):
                                mm((mt, nb), wb, rhs_b, ko, False,
                                   ko == KO_H - 1)
                else:
                    for ko in range(KO_IN):
                        for u in units:
                            mm(u, wa, rhs_a, ko, ko == 0, False)
                    for ko in range(KO_H):
                        for u in units:
                            mm(u, wb, rhs_b, ko, False, ko == KO_H - 1)
                for u in units:
                    consume(*u, ps[u])

                # steady tail: mt-wise, one PSUM bank at a time
                for mt in range(BLK, MT):
                    wa_t = w_tile(wa_ap, mt, f"wa{mt}")
                    wb_t = w_tile(wb_ap, mt, f"wb{mt}")
                    for nb in range(NB):
                        ps_t = ps_pool.tile(
                            [P, NFREE], F32, tag="ps", name=f"ps_{mt}_{nb}")
                        nbs = slice(nb * NFREE, (nb + 1) * NFREE)
                        for ko in range(KO_IN):
                            nc.tensor.matmul(
                                ps_t[:],
                                wa_t[:, ko * P:(ko + 1) * P],
                                rhs_a[:, ko, nbs],
                                start=(ko == 0), stop=False,
                            )
                        for ko in range(KO_H):
                            nc.tensor.matmul(
                                ps_t[:],
                                wb_t[:, ko * P:(ko + 1) * P],
                                rhs_b[:, ko, nbs],
                                start=False, stop=(ko == KO_H - 1),
                            )
                        consume(mt, nb, ps_t)

            # ---- phase 1: r = sigmoid(x@W_ir^T + b_ir + h@W_hr^T); rh = r*h
            def consume_r(mt, nb, ps_t):
                nbs = slice(nb * NFREE, (nb + 1) * NFREE)
                r_t = g_pool.tile([P, NFREE], F16, tag="g", name="r_t")
                nc.scalar.activation(
                    r_t[:], ps_t[:], AFT.Sigmoid,
                    bias=bias_sb["b_ir"][:, mt:mt + 1],
                )
                nc.gpsimd.dma_start(outs["rT"][mt][:, nbs], r_t[:])
                nc.vector.tensor_mul(
                    rh_sb[:, mt, nbs], r_t[:], h_sb[:, mt, nbs])

            phase(w["w_ir"], w["w_hr"], x_sb, h_sb, consume_r, True,
                  preload=(pre1a, pre1b))

            # ---- phase 2: n = tanh(x@W_in^T + b_in + rh@W_hn^T)
            # Head-block weights queued immediately so the phase's x-side
            # matmuls can start while the last rh tiles are still being
            # produced by phase 1's consumes.
            pre2a = {mt: w_tile(w["w_in"], mt, f"wa{mt}") for mt in range(BLK)}
            pre2b = {mt: w_tile(w["w_hn"], mt, f"wb{mt}") for mt in range(BLK)}

            def consume_n(mt, nb, ps_t):
                nbs = slice(nb * NFREE, (nb + 1) * NFREE)
                nc.scalar.activation(
                    n_sb[:, mt, nbs], ps_t[:], AFT.Tanh,
                    bias=bias_sb["b_in"][:, mt:mt + 1],
                )
                nc.gpsimd.dma_start(outs["nT"][mt][:, nbs], n_sb[:, mt, nbs])

            phase(w["w_in"], w["w_hn"], x_sb, rh_sb, consume_n, False,
                  preload=(pre2a, pre2b))

            # ---- phase 3: z = sigmoid(x@W_iz^T + b_iz + h@W_hz^T)
            #      h_t = n + z*(h - n)
            pre3a = {mt: w_tile(w["w_iz"], mt, f"wa{mt}") for mt in range(BLK)}
            pre3b = {mt: w_tile(w["w_hz"], mt, f"wb{mt}") for mt in range(BLK)}

            def consume_z(mt, nb, ps_t):
                nbs = slice(nb * NFREE, (nb + 1) * NFREE)
                z_t = g_pool.tile([P, NFREE], F16, tag="g", name="z_t")
                nc.scalar.activation(
                    z_t[:], ps_t[:], AFT.Sigmoid,
                    bias=bias_sb["b_iz"][:, mt:mt + 1],
                )
                nc.gpsimd.dma_start(outs["zT"][mt][:, nbs], z_t[:])
                d_t = g_pool.tile([P, NFREE], F16, tag="g", name="d_t")
                nc.vector.tensor_sub(
                    d_t[:], h_sb[:, mt, nbs], n_sb[:, mt, nbs])
                nc.vector.tensor_mul(d_t[:], z_t[:], d_t[:])
                ht_t = g_pool.tile([P, NFREE], F16, tag="g", name="ht_t")
                nc.vector.tensor_add(ht_t[:], n_sb[:, mt, nbs], d_t[:])
                nc.gpsimd.dma_start(outs["htT"][mt][:, nbs], ht_t[:])

            phase(w["w_iz"], w["w_hz"], x_sb, h_sb, consume_z, False,
                  preload=(pre3a, pre3b))

    nc.finalize()
    return nc


_NC = None


def _get_nc():
    global _NC
    if _NC is None:
        _NC = _build_nc()
    return _NC


def _pack_w(W):
    # W [H, IN] -> [MT, P, KO*P] with W_host[mt, p, ko, m] = W[mt*P+m, ko*P+p]
    W = np.ascontiguousarray(np.asarray(W, dtype=np.float32))
    return np.ascontiguousarray(
        W.reshape(MT, P, KO_IN, P).transpose(0, 3, 2, 1)
    ).reshape(MT, P, KO_IN * P).astype(np.float16)


def _pack_act(a):
    # a [BS, D] -> [P, KO, BS] with a_host[p, ko, b] = a[b, ko*P+p]
    return np.ascontiguousarray(
        np.asarray(a, dtype=np.float32).reshape(BS, -1, P).transpose(2, 1, 0)
    ).astype(np.float16)


def _pack_b(bvec):
    # b [H] -> [P, MT] with b_host[p, mt] = b[mt*P+p]
    return np.ascontiguousarray(
        np.asarray(bvec, dtype=np.float32).reshape(MT, P).T
    )


def _unpack(arr):
    # [MT, P, BS] fp16 -> [BS, H] fp32
    return np.ascontiguousarray(
        arr.transpose(2, 0, 1).astype(np.float32)
    ).reshape(BS, H)


def kernel(x, h, W_ir, b_ir, W_hr, W_iz, b_iz, W_hz, W_in, b_in, W_hn):
    global LAST_RESULTS
    nc = _get_nc()

    x = np.ascontiguousarray(np.asarray(x, dtype=np.float32))
    h = np.ascontiguousarray(np.asarray(h, dtype=np.float32))

    shared = {
        "w_ir": _pack_w(W_ir), "w_hr": _pack_w(W_hr),
        "w_iz": _pack_w(W_iz), "w_hz": _pack_w(W_hz),
        "w_in": _pack_w(W_in), "w_hn": _pack_w(W_hn),
        "b_ir": _pack_b(b_ir), "b_iz": _pack_b(b_iz), "b_in": _pack_b(b_in),
    }
    in_maps = []
    for c in range(N_CORES):
        sl = slice(c * BS, (c + 1) * BS)
        in_maps.append({
            "xT": _pack_act(x[sl]),
            "hT": _pack_act(h[sl]),
            **shared,
        })

    res = run_bass_kernel_spmd(
        nc, in_maps, core_ids=list(range(N_CORES)), trace=TRACE
    )
    LAST_RESULTS = res

    def full(name):
        return np.concatenate(
            [_unpack(res.results[c][name]) for c in range(N_CORES)], axis=0
        )

    return full("htT"), full("rT"), full("zT"), full("nT")


# revision 10
# speedup vs baseline: 1.0100x; 1.0100x over previous
"""Trainium2 Bass kernel for a CustomGRUCell.

reference:
    r = sigmoid(x @ W_ir.T + b_ir + h @ W_hr.T)
    z = sigmoid(x @ W_iz.T + b_iz + h @ W_hz.T)
    n = tanh(x @ W_in.T + b_in + (r * h) @ W_hn.T)
    h_t = (1 - z) * n + z * h
    returns (h_t, r, z, n)

Shapes: x,h [8192, 2048]; W_* [2048, 2048]; b_* [2048]. All float32.

Strategy: data-parallel over the batch dim (1024 rows per core, 8 cores),
weights replicated. All compute happens in the "transposed world":
the host packs x^T, h^T and W^T so the contraction dim (IN / H-col) lands
on SBUF partitions for both matmul operands; outputs come back as
gate^T [H, B_shard] and are untransposed on the host.

All matmul operands are float16 (same 1 cycle/row PE rate as fp32r at
moving>=256, half the HBM traffic, and a 10-bit mantissa that keeps the
quantization error ~2e-4 rel). PSUM accumulates fp32. x^T, h^T, r*h and
n all stay resident in SBUF (fp16 halves their footprint), so there are
no DRAM scratch roundtrips at all. Outputs are written fp16 and upcast
to fp32 on the host.

Per-core device schedule (M-tile = 128 rows of H, N chunk = 512 batch
cols, K subtile = 128):
  warmup:      short dummy-matmul accumulation chain issued before any
               input DMA lands, so the PE p-state ramp happens while
               waiting for data instead of during the first real tiles.
  phase 1 (r): psum = sum_k W_ir^T[k,m] x^T[k,n] + sum_k W_hr^T h^T
               r = sigmoid(psum + b_ir)  -> DRAM (fp16)
               rh = r * h^T              -> rh_sb (SBUF, fp16)
  phase 2 (n): psum = x-gemm + rh-gemm; n = tanh(psum + b_in)
               -> n_sb (SBUF, fp16) -> DRAM (fp16)
  phase 3 (z): psum = x-gemm + h-gemm;  z = sigmoid(psum + b_iz) -> DRAM
               h_t = n + z*(h - n)       -> DRAM (all operands in SBUF)
"""

import numpy as np

import concourse.bass as bass
import concourse.bacc as bacc
import concourse.mybir as mybir
import concourse.tile as tile
from concourse.bass_utils import run_bass_kernel_spmd

F32 = mybir.dt.float32
F16 = mybir.dt.float16
AFT = mybir.ActivationFunctionType

# Problem constants (hardcoded per contract).
B_FULL = 8192
IN = 2048
H = 2048
N_CORES = 8
BS = B_FULL // N_CORES  # 1024 batch rows per core
P = 128
KO_IN = IN // P  # 16 contraction subtiles for x-gemms
KO_H = H // P    # 16 contraction subtiles for h/rh-gemms
MT = H // P      # 16 output row tiles
NFREE = 512      # moving free dim per matmul (1 PSUM bank of fp32)
NB = BS // NFREE  # 2 batch chunks per core
WARM_MMS = 2     # dummy matmuls bridging the gap until the first x/w DMA lands

# Set by the test harness to capture an NTFF profile.
TRACE = False
LAST_RESULTS = None


def _build_nc():
    nc = bacc.Bacc("TRN2", target_bir_lowering=False, debug=False)

    xT = nc.dram_tensor("xT", [P, KO_IN, BS], F16, kind="ExternalInput").ap()
    hT = nc.dram_tensor("hT", [P, KO_H, BS], F16, kind="ExternalInput").ap()
    w = {
        name: nc.dram_tensor(name, [MT, P, KO_IN * P], F16, kind="ExternalInput").ap()
        for name in ("w_ir", "w_hr", "w_iz", "w_hz", "w_in", "w_hn")
    }
    b = {
        name: nc.dram_tensor(name, [P, MT], F32, kind="ExternalInput").ap()
        for name in ("b_ir", "b_iz", "b_in")
    }
    outs = {
        name: nc.dram_tensor(name, [MT, P, BS], F16, kind="ExternalOutput").ap()
        for name in ("rT", "zT", "nT", "htT")
    }

    with tile.TileContext(nc) as tc:
        with (
            tc.tile_pool(name="xres", bufs=1) as x_pool,
            tc.tile_pool(name="hres", bufs=1) as h_pool,
            tc.tile_pool(name="rhres", bufs=1) as rh_pool,
            tc.tile_pool(name="nres", bufs=1) as n_pool,
            tc.tile_pool(name="warm", bufs=1) as warm_pool,
            tc.tile_pool(name="wstream", bufs=8) as w_pool,
            tc.tile_pool(name="gates", bufs=8) as g_pool,
            tc.tile_pool(name="bias", bufs=1) as b_pool,
            tc.tile_pool(name="psum", bufs=8, space="PSUM") as ps_pool,
        ):
            # PE warmup: a dummy accumulation chain on zeroed tiles keeps
            # the tensor engine continuously busy through its p-state ramp
            # while the first input DMAs are still in flight.
            warm_l = warm_pool.tile([P, P], F16, tag="wl")
            warm_r = warm_pool.tile([P, NFREE], F16, tag="wr")
            nc.vector.memset(warm_l[:], 0.0)
            nc.vector.memset(warm_r[:], 0.0)
            warm_ps = ps_pool.tile([P, NFREE], F32, tag="ps", name="ps_warm")
            for i in range(WARM_MMS):
                nc.tensor.matmul(
                    warm_ps[:], warm_l[:], warm_r[:],
                    start=(i == 0), stop=(i == WARM_MMS - 1),
                )

            BLK = 3  # m-tiles in the streaming head block of each phase

            def w_tile(w_ap, mt, nm, eng=None):
                t = w_pool.tile([P, KO_IN * P], F16, tag="w", name=nm)
                (eng or nc.sync).dma_start(t[:], w_ap[mt])
                return t

            # x^T / h^T stay resident in SBUF for all three phases.
            # Two hardware DMA rings: sync carries every weight strip (in
            # the exact order the PE consumes them), scalar carries the
            # x/h streams, biases, and later all output writes. Phase 1's
            # head runs its whole x-side first, so the PE only needs x0 +
            # the first W_ir chunks to start; the head strips are fetched
            # in halves so all three A strips' low-ko chunks land first.
            x_sb = x_pool.tile([P, KO_IN, BS], F16, tag="x")
            h_sb = h_pool.tile([P, KO_H, BS], F16, tag="h")
            rh_sb = rh_pool.tile([P, KO_H, BS], F16, tag="rh")
            n_sb = n_pool.tile([P, KO_H, BS], F16, tag="n")

            nc.sync.dma_start(x_sb[:, 0, :], xT[:, 0, :])
            HW = KO_IN // 2 * P  # half-strip width in elements
            pre1a = {mt: w_pool.tile([P, KO_IN * P], F16, tag="w",
                                     name=f"wa{mt}") for mt in range(BLK)}
            pre1b = {mt: w_pool.tile([P, KO_IN * P], F16, tag="w",
                                     name=f"wb{mt}") for mt in range(BLK)}
            for pre, w_ap in ((pre1a, w["w_ir"]), (pre1b, w["w_hr"])):
                for half in range(2):
                    hs = slice(half * HW, (half + 1) * HW)
                    for mt in range(BLK):
                        nc.sync.dma_start(pre[mt][:, hs], w_ap[mt][:, hs])
            for ko in range(1, KO_IN):
                nc.scalar.dma_start(x_sb[:, ko, :], xT[:, ko, :])
            for ko in range(KO_H):
                nc.scalar.dma_start(h_sb[:, ko, :], hT[:, ko, :])

            bias_sb = {}
            for name in ("b_ir", "b_iz", "b_in"):
                t = b_pool.tile([P, MT], F32, tag=name)
                nc.scalar.dma_start(t[:], b[name][:])
                bias_sb[name] = t

            def phase(wa_ap, wb_ap, rhs_a, rhs_b, consume, preload=None):
                """Head block (first BLK m-tiles): ko-loop OUTER, all
                A-side matmuls before any B-side, so the PE consumes
                streaming rhs/weight tiles in DMA arrival order and the
                B operand (h / rh) has the whole A sweep of slack to
                arrive. Remaining m-tiles: mt-wise, nb INNER so each
                loaded weight chunk is used by two consecutive matmuls."""
                mts = list(range(0, BLK))
                units = [(mt, nb) for mt in mts for nb in range(NB)]
                wa, wb = preload if preload else ({}, {})
                ps = {
                    u: ps_pool.tile(
                        [P, NFREE], F32, tag="ps", name=f"ps_{u[0]}_{u[1]}")
                    for u in units
                }

                def mm(u, w_t, rhs, ko, start, stop):
                    mt, nb = u
                    nc.tensor.matmul(
                        ps[u][:],
                        w_t[mt][:, ko * P:(ko + 1) * P],
                        rhs[:, ko, nb * NFREE:(nb + 1) * NFREE],
                        start=start,
                        stop=stop,
                    )

                for ko in range(KO_IN):
                    for u in units:
                        mm(u, wa, rhs_a, ko, ko == 0, False)
                for ko in range(KO_H):
                    for u in units:
                        mm(u, wb, rhs_b, ko, False, ko == KO_H - 1)
                for u in units:
                    consume(*u, ps[u])

                # steady tail: mt-wise, two PSUM banks per mt, ko outer
                for mt in range(BLK, MT):
                    wa_t = w_tile(wa_ap, mt, f"wa{mt}")
                    wb_t = w_tile(wb_ap, mt, f"wb{mt}")
                    ps_t = {
                        nb: ps_pool.tile(
                            [P, NFREE], F32, tag="ps", name=f"ps_{mt}_{nb}")
                        for nb in range(NB)
                    }
                    for ko in range(KO_IN):
                        for nb in range(NB):
                            nc.tensor.matmul(
                                ps_t[nb][:],
                                wa_t[:, ko * P:(ko + 1) * P],
                                rhs_a[:, ko, nb * NFREE:(nb + 1) * NFREE],
                                start=(ko == 0), stop=False,
                            )
                    for ko in range(KO_H):
                        for nb in range(NB):
                            nc.tensor.matmul(
                                ps_t[nb][:],
                                wb_t[:, ko * P:(ko + 1) * P],
                                rhs_b[:, ko, nb * NFREE:(nb + 1) * NFREE],
                                start=False, stop=(ko == KO_H - 1),
                            )
                    for nb in range(NB):
                        consume(mt, nb, ps_t[nb])

            # ---- phase 1: r = sigmoid(x@W_ir^T + b_ir + h@W_hr^T); rh = r*h
            def consume_r(mt, nb, ps_t):
                nbs = slice(nb * NFREE, (nb + 1) * NFREE)
                r_t = g_pool.tile([P, NFREE], F16, tag="g", name="r_t")
                nc.scalar.activation(
                    r_t[:], ps_t[:], AFT.Sigmoid,
                    bias=bias_sb["b_ir"][:, mt:mt + 1],
                )
                nc.scalar.dma_start(outs["rT"][mt][:, nbs], r_t[:])
                nc.vector.tensor_mul(
                    rh_sb[:, mt, nbs], r_t[:], h_sb[:, mt, nbs])

            phase(w["w_ir"], w["w_hr"], x_sb, h_sb, consume_r,
                  preload=(pre1a, pre1b))

            # ---- phase 2: n = tanh(x@W_in^T + b_in + rh@W_hn^T)
            # Head-block weights queued immediately so the phase's x-side
            # matmuls can start while the last rh tiles are still being
            # produced by phase 1's consumes.
            pre2a = {mt: w_tile(w["w_in"], mt, f"wa{mt}") for mt in range(BLK)}
            pre2b = {mt: w_tile(w["w_hn"], mt, f"wb{mt}") for mt in range(BLK)}

            def consume_n(mt, nb, ps_t):
                nbs = slice(nb * NFREE, (nb + 1) * NFREE)
                nc.scalar.activation(
                    n_sb[:, mt, nbs], ps_t[:], AFT.Tanh,
                    bias=bias_sb["b_in"][:, mt:mt + 1],
                )
                nc.scalar.dma_start(outs["nT"][mt][:, nbs], n_sb[:, mt, nbs])

            phase(w["w_in"], w["w_hn"], x_sb, rh_sb, consume_n,
                  preload=(pre2a, pre2b))

            # ---- phase 3: z = sigmoid(x@W_iz^T + b_iz + h@W_hz^T)
            #      h_t = n + z*(h - n)
            pre3a = {mt: w_tile(w["w_iz"], mt, f"wa{mt}") for mt in range(BLK)}
            pre3b = {mt: w_tile(w["w_hz"], mt, f"wb{mt}") for mt in range(BLK)}

            def consume_z(mt, nb, ps_t):
                nbs = slice(nb * NFREE, (nb + 1) * NFREE)
                z_t = g_pool.tile([P, NFREE], F16, tag="g", name="z_t")
                nc.scalar.activation(
                    z_t[:], ps_t[:], AFT.Sigmoid,
                    bias=bias_sb["b_iz"][:, mt:mt + 1],
                )
                nc.scalar.dma_start(outs["zT"][mt][:, nbs], z_t[:])
                d_t = g_pool.tile([P, NFREE], F16, tag="g", name="d_t")
                nc.vector.tensor_sub(
                    d_t[:], h_sb[:, mt, nbs], n_sb[:, mt, nbs])
                nc.vector.tensor_mul(d_t[:], z_t[:], d_t[:])
                ht_t = g_pool.tile([P, NFREE], F16, tag="g", name="ht_t")
                nc.vector.tensor_add(ht_t[:], n_sb[:, mt, nbs], d_t[:])
                nc.scalar.dma_start(outs["htT"][mt][:, nbs], ht_t[:])

            phase(w["w_iz"], w["w_hz"], x_sb, h_sb, consume_z,
                  preload=(pre3a, pre3b))

    nc.finalize()
    return nc


_NC = None


def _get_nc():
    global _NC
    if _NC is None:
        _NC = _build_nc()
    return _NC


def _pack_w(W):
    # W [H, IN] -> [MT, P, KO*P] with W_host[mt, p, ko, m] = W[mt*P+m, ko*P+p]
    W = np.ascontiguousarray(np.asarray(W, dtype=np.float32))
    return np.ascontiguousarray(
        W.reshape(MT, P, KO_IN, P).transpose(0, 3, 2, 1)
    ).reshape(MT, P, KO_IN * P).astype(np.float16)


def _pack_act(a):
    # a [BS, D] -> [P, KO, BS] with a_host[p, ko, b] = a[b, ko*P+p]
    return np.ascontiguousarray(
        np.asarray(a, dtype=np.float32).reshape(BS, -1, P).transpose(2, 1, 0)
    ).astype(np.float16)


def _pack_b(bvec):
    # b [H] -> [P, MT] with b_host[p, mt] = b[mt*P+p]
    return np.ascontiguousarray(
        np.asarray(bvec, dtype=np.float32).reshape(MT, P).T
    )


def _unpack(arr):
    # [MT, P, BS] fp16 -> [BS, H] fp32
    return np.ascontiguousarray(
        arr.transpose(2, 0, 1).astype(np.float32)
    ).reshape(BS, H)


def kernel(x, h, W_ir, b_ir, W_hr, W_iz, b_iz, W_hz, W_in, b_in, W_hn):
    global LAST_RESULTS
    nc = _get_nc()

    x = np.ascontiguousarray(np.asarray(x, dtype=np.float32))
    h = np.ascontiguousarray(np.asarray(h, dtype=np.float32))

    shared = {
        "w_ir": _pack_w(W_ir), "w_hr": _pack_w(W_hr),
        "w_iz": _pack_w(W_iz), "w_hz": _pack_w(W_hz),
        "w_in": _pack_w(W_in), "w_hn": _pack_w(W_hn),
        "b_ir": _pack_b(b_ir), "b_iz": _pack_b(b_iz), "b_in": _pack_b(b_in),
    }
    in_maps = []
    for c in range(N_CORES):
        sl = slice(c * BS, (c + 1) * BS)
        in_maps.append({
            "xT": _pack_act(x[sl]),
            "hT": _pack_act(h[sl]),
            **shared,
        })

    res = run_bass_kernel_spmd(
        nc, in_maps, core_ids=list(range(N_CORES)), trace=TRACE
    )
    LAST_RESULTS = res

    def full(name):
        return np.concatenate(
            [_unpack(res.results[c][name]) for c in range(N_CORES)], axis=0
        )

    return full("htT"), full("rT"), full("zT"), full("nT")


# revision 13
# speedup vs baseline: 1.0101x; 1.0001x over previous
"""Trainium2 Bass kernel for a CustomGRUCell.

reference:
    r = sigmoid(x @ W_ir.T + b_ir + h @ W_hr.T)
    z = sigmoid(x @ W_iz.T + b_iz + h @ W_hz.T)
    n = tanh(x @ W_in.T + b_in + (r * h) @ W_hn.T)
    h_t = (1 - z) * n + z * h
    returns (h_t, r, z, n)

Shapes: x,h [8192, 2048]; W_* [2048, 2048]; b_* [2048]. All float32.

Strategy: data-parallel over the batch dim (1024 rows per core, 8 cores),
weights replicated. All compute happens in the "transposed world":
the host packs x^T, h^T and W^T so the contraction dim (IN / H-col) lands
on SBUF partitions for both matmul operands; outputs come back as
gate^T [H, B_shard] and are untransposed on the host.

All matmul operands are float16 (same 1 cycle/row PE rate as fp32r at
moving>=256, half the HBM traffic, and a 10-bit mantissa that keeps the
quantization error ~2e-4 rel). PSUM accumulates fp32. x^T, h^T, r*h and
n all stay resident in SBUF (fp16 halves their footprint), so there are
no DRAM scratch roundtrips at all. Outputs are written fp16 and upcast
to fp32 on the host.

Per-core device schedule (M-tile = 128 rows of H, N chunk = 512 batch
cols, K subtile = 128):
  warmup:      short dummy-matmul accumulation chain issued before any
               input DMA lands, so the PE p-state ramp happens while
               waiting for data instead of during the first real tiles.
  phase 1 (r): psum = sum_k W_ir^T[k,m] x^T[k,n] + sum_k W_hr^T h^T
               r = sigmoid(psum + b_ir)  -> DRAM (fp16)
               rh = r * h^T              -> rh_sb (SBUF, fp16)
  phase 2 (n): psum = x-gemm + rh-gemm; n = tanh(psum + b_in)
               -> n_sb (SBUF, fp16) -> DRAM (fp16)
  phase 3 (z): psum = x-gemm + h-gemm;  z = sigmoid(psum + b_iz) -> DRAM
               h_t = n + z*(h - n)       -> DRAM (all operands in SBUF)
"""

import numpy as np

import concourse.bass as bass
import concourse.bacc as bacc
import concourse.mybir as mybir
import concourse.tile as tile
from concourse.bass_utils import run_bass_kernel_spmd

F32 = mybir.dt.float32
F16 = mybir.dt.float16
BF16 = mybir.dt.bfloat16
AFT = mybir.ActivationFunctionType

# Problem constants (hardcoded per contract).
B_FULL = 8192
IN = 2048
H = 2048
N_CORES = 8
BS = B_FULL // N_CORES  # 1024 batch rows per core
P = 128
KO_IN = IN // P  # 16 contraction subtiles for x-gemms
KO_H = H // P    # 16 contraction subtiles for h/rh-gemms
MT = H // P      # 16 output row tiles
NFREE = 512      # moving free dim per matmul (1 PSUM bank of fp32)
NB = BS // NFREE  # 2 batch chunks per core
WARM_MMS = 6     # dummy matmuls bridging the gap until the first x/w DMA lands

# Set by the test harness to capture an NTFF profile.
TRACE = False
LAST_RESULTS = None


def _build_nc():
    nc = bacc.Bacc("TRN2", target_bir_lowering=False, debug=False)

    xT = nc.dram_tensor("xT", [P, KO_IN, BS], F16, kind="ExternalInput").ap()
    hT = nc.dram_tensor("hT", [P, KO_H, BS], F16, kind="ExternalInput").ap()
    w = {
        name: nc.dram_tensor(name, [MT, P, KO_IN * P], BF16, kind="ExternalInput").ap()
        for name in ("w_ir", "w_hr", "w_iz", "w_hz", "w_in", "w_hn")
    }
    b = {
        name: nc.dram_tensor(name, [P, MT], F32, kind="ExternalInput").ap()
        for name in ("b_ir", "b_iz", "b_in")
    }
    outs = {
        name: nc.dram_tensor(name, [MT, P, BS], F16, kind="ExternalOutput").ap()
        for name in ("rT", "zT", "nT", "htT")
    }

    with tile.TileContext(nc) as tc:
        with (
            tc.tile_pool(name="xres", bufs=1) as x_pool,
            tc.tile_pool(name="hres", bufs=1) as h_pool,
            tc.tile_pool(name="rhres", bufs=1) as rh_pool,
            tc.tile_pool(name="nres", bufs=1) as n_pool,
            tc.tile_pool(name="warm", bufs=1) as warm_pool,
            tc.tile_pool(name="wstream", bufs=8) as w_pool,
            tc.tile_pool(name="gates", bufs=8) as g_pool,
            tc.tile_pool(name="bias", bufs=1) as b_pool,
            tc.tile_pool(name="psum", bufs=8, space="PSUM") as ps_pool,
        ):
            # PE warmup: a dummy accumulation chain on zeroed tiles keeps
            # the tensor engine continuously busy through its p-state ramp
            # while the first input DMAs are still in flight.
            warm_l = warm_pool.tile([P, P], BF16, tag="wl")
            warm_r = warm_pool.tile([P, NFREE], F16, tag="wr")
            nc.vector.memset(warm_l[:], 0.0)
            nc.vector.memset(warm_r[:], 0.0)
            warm_ps = ps_pool.tile([P, NFREE], F32, tag="ps", name="ps_warm")
            for i in range(WARM_MMS):
                nc.tensor.matmul(
                    warm_ps[:], warm_l[:], warm_r[:],
                    start=(i == 0), stop=(i == WARM_MMS - 1),
                )

            BLK = 3  # m-tiles in the streaming head block of each phase

            def w_tile(w_ap, mt, nm, eng=None):
                t = w_pool.tile([P, KO_IN * P], BF16, tag="w", name=nm)
                (eng or nc.sync).dma_start(t[:], w_ap[mt])
                return t

            # x^T / h^T stay resident in SBUF for all three phases.
            # Two hardware DMA rings: sync carries every weight strip (in
            # the exact order the PE consumes them), scalar carries the
            # x/h streams, biases, and later all output writes. Phase 1's
            # head runs its whole x-side first, so the PE only needs x0 +
            # the first W_ir chunks to start; the head strips are fetched
            # in halves so all three A strips' low-ko chunks land first.
            x_sb = x_pool.tile([P, KO_IN, BS], F16, tag="x")
            h_sb = h_pool.tile([P, KO_H, BS], F16, tag="h")
            rh_sb = rh_pool.tile([P, KO_H, BS], F16, tag="rh")
            n_sb = n_pool.tile([P, KO_H, BS], F16, tag="n")

            nc.sync.dma_start(x_sb[:, 0, :], xT[:, 0, :])
            HW = KO_IN // 2 * P  # half-strip width in elements
            pre1a = {mt: w_pool.tile([P, KO_IN * P], BF16, tag="w",
                                     name=f"wa{mt}") for mt in range(BLK)}
            pre1b = {mt: w_pool.tile([P, KO_IN * P], BF16, tag="w",
                                     name=f"wb{mt}") for mt in range(BLK)}
            QW = KO_IN // 4 * P  # quarter-strip width in elements
            for q in range(4):
                qs = slice(q * QW, (q + 1) * QW)
                for mt in range(BLK):
                    nc.sync.dma_start(pre1a[mt][:, qs], w["w_ir"][mt][:, qs])
            for half in range(2):
                hs = slice(half * HW, (half + 1) * HW)
                for mt in range(BLK):
                    nc.sync.dma_start(pre1b[mt][:, hs], w["w_hr"][mt][:, hs])
            for ko in range(1, KO_IN):
                nc.scalar.dma_start(x_sb[:, ko, :], xT[:, ko, :])
            for ko in range(KO_H):
                nc.scalar.dma_start(h_sb[:, ko, :], hT[:, ko, :])

            bias_sb = {}
            for name in ("b_ir", "b_iz", "b_in"):
                t = b_pool.tile([P, MT], F32, tag=name)
                nc.scalar.dma_start(t[:], b[name][:])
                bias_sb[name] = t

            def phase(wa_ap, wb_ap, rhs_a, rhs_b, consume, preload=None):
                """Head block (first BLK m-tiles): ko-loop OUTER, all
                A-side matmuls before any B-side, so the PE consumes
                streaming rhs/weight tiles in DMA arrival order and the
                B operand (h / rh) has the whole A sweep of slack to
                arrive. Remaining m-tiles: mt-wise, nb INNER so each
                loaded weight chunk is used by two consecutive matmuls."""
                mts = list(range(0, BLK))
                units = [(mt, nb) for mt in mts for nb in range(NB)]
                wa, wb = preload if preload else ({}, {})
                ps = {
                    u: ps_pool.tile(
                        [P, NFREE], F32, tag="ps", name=f"ps_{u[0]}_{u[1]}")
                    for u in units
                }

                def mm(u, w_t, rhs, ko, start, stop):
                    mt, nb = u
                    nc.tensor.matmul(
                        ps[u][:],
                        w_t[mt][:, ko * P:(ko + 1) * P],
                        rhs[:, ko, nb * NFREE:(nb + 1) * NFREE],
                        start=start,
                        stop=stop,
                    )

                for ko in range(KO_IN):
                    for u in units:
                        mm(u, wa, rhs_a, ko, ko == 0, False)
                for ko in range(KO_H):
                    for u in units:
                        mm(u, wb, rhs_b, ko, False, ko == KO_H - 1)
                for u in units:
                    consume(*u, ps[u])

                # steady tail: mt-wise, two PSUM banks per mt, ko outer
                for mt in range(BLK, MT):
                    wa_t = w_tile(wa_ap, mt, f"wa{mt}")
                    wb_t = w_tile(wb_ap, mt, f"wb{mt}")
                    ps_t = {
                        nb: ps_pool.tile(
                            [P, NFREE], F32, tag="ps", name=f"ps_{mt}_{nb}")
                        for nb in range(NB)
                    }
                    for ko in range(KO_IN):
                        for nb in range(NB):
                            nc.tensor.matmul(
                                ps_t[nb][:],
                                wa_t[:, ko * P:(ko + 1) * P],
                                rhs_a[:, ko, nb * NFREE:(nb + 1) * NFREE],
                                start=(ko == 0), stop=False,
                            )
                    for ko in range(KO_H):
                        for nb in range(NB):
                            nc.tensor.matmul(
                                ps_t[nb][:],
                                wb_t[:, ko * P:(ko + 1) * P],
                                rhs_b[:, ko, nb * NFREE:(nb + 1) * NFREE],
                                start=False, stop=(ko == KO_H - 1),
                            )
                    for nb in range(NB):
                        consume(mt, nb, ps_t[nb])

            # ---- phase 1: r = sigmoid(x@W_ir^T + b_ir + h@W_hr^T); rh = r*h
            def consume_r(mt, nb, ps_t):
                nbs = slice(nb * NFREE, (nb + 1) * NFREE)
                r_t = g_pool.tile([P, NFREE], F16, tag="g", name="r_t")
                nc.scalar.activation(
                    r_t[:], ps_t[:], AFT.Sigmoid,
                    bias=bias_sb["b_ir"][:, mt:mt + 1],
                )
                nc.scalar.dma_start(outs["rT"][mt][:, nbs], r_t[:])
                nc.vector.tensor_mul(
                    rh_sb[:, mt, nbs], r_t[:], h_sb[:, mt, nbs])

            phase(w["w_ir"], w["w_hr"], x_sb, h_sb, consume_r,
                  preload=(pre1a, pre1b))

            # ---- phase 2: n = tanh(x@W_in^T + b_in + rh@W_hn^T)
            # Head-block weights queued immediately so the phase's x-side
            # matmuls can start while the last rh tiles are still being
            # produced by phase 1's consumes.
            pre2a = {mt: w_tile(w["w_in"], mt, f"wa{mt}") for mt in range(BLK)}
            pre2b = {mt: w_tile(w["w_hn"], mt, f"wb{mt}") for mt in range(BLK)}

            def consume_n(mt, nb, ps_t):
                nbs = slice(nb * NFREE, (nb + 1) * NFREE)
                nc.scalar.activation(
                    n_sb[:, mt, nbs], ps_t[:], AFT.Tanh,
                    bias=bias_sb["b_in"][:, mt:mt + 1],
                )
                nc.scalar.dma_start(outs["nT"][mt][:, nbs], n_sb[:, mt, nbs])

            phase(w["w_in"], w["w_hn"], x_sb, rh_sb, consume_n,
                  preload=(pre2a, pre2b))

            # ---- phase 3: z = sigmoid(x@W_iz^T + b_iz + h@W_hz^T)
            #      h_t = n + z*(h - n)
            pre3a = {mt: w_tile(w["w_iz"], mt, f"wa{mt}") for mt in range(BLK)}
            pre3b = {mt: w_tile(w["w_hz"], mt, f"wb{mt}") for mt in range(BLK)}

            def consume_z(mt, nb, ps_t):
                # Final m-tiles dispatch their output DMAs on the (by then
                # idle) sync ring so the last activation isn't queued
                # behind dispatch instructions on the scalar engine.
                eng = nc.sync if mt >= MT - 2 else nc.scalar
                nbs = slice(nb * NFREE, (nb + 1) * NFREE)
                z_t = g_pool.tile([P, NFREE], F16, tag="g", name="z_t")
                nc.scalar.activation(
                    z_t[:], ps_t[:], AFT.Sigmoid,
                    bias=bias_sb["b_iz"][:, mt:mt + 1],
                )
                eng.dma_start(outs["zT"][mt][:, nbs], z_t[:])
                d_t = g_pool.tile([P, NFREE], F16, tag="g", name="d_t")
                nc.vector.tensor_sub(
                    d_t[:], h_sb[:, mt, nbs], n_sb[:, mt, nbs])
                nc.vector.tensor_mul(d_t[:], z_t[:], d_t[:])
                ht_t = g_pool.tile([P, NFREE], F16, tag="g", name="ht_t")
                nc.vector.tensor_add(ht_t[:], n_sb[:, mt, nbs], d_t[:])
                eng.dma_start(outs["htT"][mt][:, nbs], ht_t[:])

            phase(w["w_iz"], w["w_hz"], x_sb, h_sb, consume_z,
                  preload=(pre3a, pre3b))

    nc.finalize()
    return nc


_NC = None


def _get_nc():
    global _NC
    if _NC is None:
        _NC = _build_nc()
    return _NC


def _pack_w(W):
    # W [H, IN] -> [MT, P, KO*P] with W_host[mt, p, ko, m] = W[mt*P+m, ko*P+p]
    import ml_dtypes
    W = np.ascontiguousarray(np.asarray(W, dtype=np.float32))
    return np.ascontiguousarray(
        W.reshape(MT, P, KO_IN, P).transpose(0, 3, 2, 1)
    ).reshape(MT, P, KO_IN * P).astype(ml_dtypes.bfloat16)


def _pack_act(a):
    # a [BS, D] -> [P, KO, BS] with a_host[p, ko, b] = a[b, ko*P+p]
    return np.ascontiguousarray(
        np.asarray(a, dtype=np.float32).reshape(BS, -1, P).transpose(2, 1, 0)
    ).astype(np.float16)


def _pack_b(bvec):
    # b [H] -> [P, MT] with b_host[p, mt] = b[mt*P+p]
    return np.ascontiguousarray(
        np.asarray(bvec, dtype=np.float32).reshape(MT, P).T
    )


def _unpack(arr):
    # [MT, P, BS] fp16 -> [BS, H] fp32
    return np.ascontiguousarray(
        arr.transpose(2, 0, 1).astype(np.float32)
    ).reshape(BS, H)


def kernel(x, h, W_ir, b_ir, W_hr, W_iz, b_iz, W_hz, W_in, b_in, W_hn):
    global LAST_RESULTS
    nc = _get_nc()

    x = np.ascontiguousarray(np.asarray(x, dtype=np.float32))
    h = np.ascontiguousarray(np.asarray(h, dtype=np.float32))

    shared = {
        "w_ir": _pack_w(W_ir), "w_hr": _pack_w(W_hr),
        "w_iz": _pack_w(W_iz), "w_hz": _pack_w(W_hz),
        "w_in": _pack_w(W_in), "w_hn": _pack_w(W_hn),
        "b_ir": _pack_b(b_ir), "b_iz": _pack_b(b_iz), "b_in": _pack_b(b_in),
    }
    in_maps = []
    for c in range(N_CORES):
        sl = slice(c * BS, (c + 1) * BS)
        in_maps.append({
            "xT": _pack_act(x[sl]),
            "hT": _pack_act(h[sl]),
            **shared,
        })

    res = run_bass_kernel_spmd(
        nc, in_maps, core_ids=list(range(N_CORES)), trace=TRACE
    )
    LAST_RESULTS = res

    def full(name):
        return np.concatenate(
            [_unpack(res.results[c][name]) for c in range(N_CORES)], axis=0
        )

    return full("htT"), full("rT"), full("zT"), full("nT")
